# revision 1
# baseline (speedup 1.0000x reference)
"""Trainium2 Bass kernel for nn_AutomatonPELayer (n=512, k=16, d=512).

Math: the reference solves B x = tile(p) with B = I - kron(shift, T),
which is block upper-bidiagonal => x_i = p + T x_{i+1}, i.e.
stacked[i] = (sum_{j=0}^{n-1-i} T^j) p.  We compute Y[:, j] = T^j p via a
log-depth doubling scan on the tensor engine, reduce with per-core
anti-triangular 0/1 masks (matmul contraction over the sequence dim, which
also performs the index reversal), and apply the output projection
pe = stacked @ W.T + b as one fused K=17 matmul (ones row carries the bias).

Each of the 8 cores redundantly runs the tiny scan and computes its own 64
output positions; the only sharded work is the mask reduction + output
projection + output DMA.  Host side does layout-only prep (transpose W,
build 0/1 masks, concat shards).

Hardware notes shaping the code:
  - TRN2 instructions encode one semaphore wait; extra waits become EVSEM
    splits (Bacc.generate_event_semaphores), so deps are kept narrow: three
    separate input DMAs (seed/wb/mask) whose consumers each wait on one
    queue, and all PSUM->SBUF copies on DVE.
  - The seed DMA is tiny so the scan starts immediately; wb/mask arrive
    during the scan.
  - Compute-engine SBUF APs must start at partition 0/32/64/96, so P/Q are
    stacked along the free dim and the bias ones-row is made by memsetting
    the whole S tile to 1.0 before overwriting rows 0:16.
  - PSUM columns are never recycled within the kernel, so no WAR waits.
"""

import numpy as np

N = 512  # sentence length handled by the device kernel
K = 16  # num states
D = 512  # embed dim
NCORES = 8
PPOS = N // NCORES  # positions per core (64)

# seed tile layout (cols): Q1 = T^T | P1 = T | p | I
SEED_Q1 = 0
SEED_P1 = 16
SEED_P = 32
SEED_I = 48

_NC_CACHE = {}

# "raw": hand-scheduled Bacc build (default, fastest).
# "f32": TileContext build, exact fp32.
# "mixed": TileContext build, final projection in float32r (faster tail,
#          ~1e-4 relative error instead of ~2e-6).
VARIANT = "raw"

# Set by an external harness to capture a profile; grading path leaves these.
TRACE = False
LAST_RESULT = None


def _host_fallback(p, T, W, b, n):
    # Closed-form reference for shapes the compiled kernel doesn't handle.
    p = p.reshape(-1).astype(np.float64)
    T = T.astype(np.float64)
    k = p.shape[0]
    stacked = np.zeros((n, k), dtype=np.float64)
    acc = np.zeros(k, dtype=np.float64)
    for i in range(n - 1, -1, -1):
        acc = p + (T @ acc if i < n - 1 else 0.0)
        stacked[i] = acc
    pe = stacked @ W.astype(np.float64).T + b.astype(np.float64)
    return pe.astype(np.float32)


def _build_nc(variant):
    import concourse.mybir as mybir
    from concourse import bacc
    from concourse.tile import TileContext

    f32 = mybir.dt.float32
    # float32r matmuls (single-pass) are only ISA-legal at M=128 with even,
    # 8B-aligned operands; we use them for the final projection only.
    fdt = mybir.dt.float32r if variant == "mixed" else f32

    nc = bacc.Bacc("TRN2", target_bir_lowering=False)

    dSeed = nc.dram_tensor("seed", [K, 64], f32, kind="ExternalInput")
    dWb = nc.dram_tensor("wb", [K + 2, D], fdt, kind="ExternalInput")
    dMask = nc.dram_tensor("mask", [128, 4 * PPOS], f32, kind="ExternalInput")
    out_shape = [PPOS, D] if variant == "f32" else [128, 4 * PPOS]
    dOut = nc.dram_tensor("out", out_shape, f32, kind="ExternalOutput")

    with TileContext(nc) as tc:
        with (
            tc.tile_pool(name="sb", bufs=1) as sb,
            tc.tile_pool(name="ps", bufs=1, space="PSUM") as ps,
        ):
            tSeed = sb.tile([K, 64], f32, tag="Seed", name="tSeed")
            nc.sync.dma_start(out=tSeed[:], in_=dSeed[:])
            tWb = sb.tile([K + 1, D], fdt, tag="Wb", name="tWb")
            nc.sync.dma_start(out=tWb[:], in_=dWb[0 : K + 1, :])
            tMask = sb.tile([128, 4 * PPOS], f32, tag="Mask", name="tMask")
            nc.sync.dma_start(out=tMask[:], in_=dMask[:])

            tI = tSeed[:, SEED_I : SEED_I + 16]

            # S-hat: row 16 (bias ones-row) arrives by DMA from the wb
            # tensor's extra ones row; rows 0:16 come from the reduction.
            tS = sb.tile([K + 1, PPOS], fdt, tag="S", name="tS")
            nc.sync.dma_start(out=tS[K : K + 1, :], in_=dWb[K + 1 : K + 2, 0:PPOS])

            tY = sb.tile([K, 256], f32, tag="Y", name="tY")
            nc.vector.tensor_copy(out=tY[:, 0:1], in_=tSeed[:, SEED_P : SEED_P + 1])

            # --- doubling scan ---
            # tPQ_w[:, 0:16] = Q_w = (T^w)^T, tPQ_w[:, 16:32] = P_w = T^w.
            # matmul computes lhsT.T @ rhs:
            #   Q_2w = Q_w Q_w = matmul(lhsT=P_w, rhs=Q_w)
            #   P_2w = P_w P_w = matmul(lhsT=Q_w, rhs=P_w)
            #   Y[:, w:2w] = P_w Y[:, :w] = matmul(lhsT=Q_w, rhs=Y[:, :w])
            psPQ = ps.tile([K, 256], f32, tag="psPQ", name="psPQ")
            psE = ps.tile([K, 256], f32, tag="psE", name="psE")
            cur = tSeed[:, 0:32]
            pq_saved = {}
            w = 1
            r = 0
            while w <= 128:
                tQ = cur[:, 0:16]
                tP = cur[:, 16:32]
                last = w == 128
                c0 = 32 * r
                nc.tensor.matmul(
                    psPQ[:, c0 : c0 + 16], lhsT=tP, rhs=tQ, start=True, stop=True
                )
                if not last:
                    nc.tensor.matmul(
                        psPQ[:, c0 + 16 : c0 + 32],
                        lhsT=tQ,
                        rhs=tP,
                        start=True,
                        stop=True,
                    )
                nc.tensor.matmul(
                    psE[:, w : 2 * w], lhsT=tQ, rhs=tY[:, 0:w], start=True, stop=True
                )
                nxt = sb.tile([K, 32], f32, tag=f"PQ{2 * w}", name=f"tPQ{2 * w}")
                cw = 16 if last else 32
                nc.vector.tensor_copy(out=nxt[:, 0:cw], in_=psPQ[:, c0 : c0 + cw])
                nc.vector.tensor_copy(out=tY[:, w : 2 * w], in_=psE[:, w : 2 * w])
                pq_saved[2 * w] = nxt
                cur = nxt[:]
                w *= 2
                r += 1

            # --- transposed Y chunks, packed into one [128, 64] tile:
            # chunk k rows j hold y_{128k+j}^T (chunk k = Y_slice.T @ R) ---
            q128 = pq_saved[128][:, 0:16]
            q256 = pq_saved[256][:, 0:16]
            chunk_src = [
                (tY[:, 0:128], tI),
                (tY[:, 0:128], q128),
                (tY[:, 0:128], q256),
                (tY[:, 128:256], q256),
            ]
            psT = ps.tile([128, 4 * K], f32, tag="psT", name="psT")
            for kk, (lhs, rhs) in enumerate(chunk_src):
                nc.tensor.matmul(
                    psT[:, kk * K : (kk + 1) * K],
                    lhsT=lhs,
                    rhs=rhs,
                    start=True,
                    stop=True,
                )
            tYt = sb.tile([128, 4 * K], f32, tag="YtAll", name="tYt")
            nc.vector.tensor_copy(out=tYt[:], in_=psT[:])

            # --- masked reduction: S[:, t] = sum_j y_j * mask[j, t] ---
            psS = ps.tile([K, PPOS], f32, tag="psS", name="psS")
            for kk in range(4):
                nc.tensor.matmul(
                    psS[:],
                    lhsT=tYt[:, kk * K : (kk + 1) * K],
                    rhs=tMask[:, kk * PPOS : (kk + 1) * PPOS],
                    start=(kk == 0),
                    stop=(kk == 3),
                )
            nc.vector.tensor_copy(out=tS[0:K, :], in_=psS[:])

            # --- output projection, bias fused via ones row 16 of tS ---
            if variant == "f32":
                # one [64, 512] matmul: psO[t, :] = pe[c*64+t, :]
                psO = ps.tile([PPOS, D], f32, tag="psO", name="psO")
                nc.tensor.matmul(psO[:], lhsT=tS[:], rhs=tWb[:], start=True, stop=True)
                tOut = sb.tile([PPOS, D], f32, tag="outT", name="tOut")
            else:
                # transposed, M=128 so float32r is ISA-legal:
                # psO[i, e*64+t] = pe[c*64+t, e*128+i]
                psO = ps.tile([128, 4 * PPOS], f32, tag="psO", name="psO")
                for e in range(4):
                    nc.tensor.matmul(
                        psO[:, e * PPOS : (e + 1) * PPOS],
                        lhsT=tWb[:, e * 128 : (e + 1) * 128],
                        rhs=tS[:],
                        start=True,
                        stop=True,
                    )
                tOut = sb.tile([128, 4 * PPOS], f32, tag="outT", name="tOut")
            nc.vector.tensor_copy(out=tOut[:], in_=psO[:])
            nc.sync.dma_start(out=dOut[:], in_=tOut[:])

    nc.compile()
    return nc


def _build_nc_raw():
    """Hand-scheduled variant: no TileContext, explicit semaphores.

    Engine streams (each instruction carries at most one wait; the two
    unavoidable extra DMA waits ride as absorbers on otherwise-waitless
    PE instructions, which Bacc legalizes):
      SP : dma seed | dma wb | dma ones->S | dma mask | dma out | wait out
      PE : 8 rounds of (mmQ, mmP, mmE) | 4 chunk | 4 mask | final
      DVE: p-copy | 8x (PQ-copy, E-copy) | Yt | S | out-copy
    """
    from contextlib import ExitStack

    import concourse.mybir as mybir
    from concourse import bacc

    f32 = mybir.dt.float32
    nc = bacc.Bacc("TRN2", target_bir_lowering=False)

    dSeed = nc.dram_tensor("seed", [K, 64], f32, kind="ExternalInput")
    dWb = nc.dram_tensor("wb", [K + 2, D], f32, kind="ExternalInput")
    dMask = nc.dram_tensor("mask", [128, 4 * PPOS], f32, kind="ExternalInput")
    dOut = nc.dram_tensor("out", [PPOS, D], f32, kind="ExternalOutput")

    with ExitStack() as ctx:
        def sb(name, shape):
            return ctx.enter_context(nc.sbuf_tensor(name, shape, f32))

        def psb(name, shape):
            return ctx.enter_context(nc.psum_tensor(name, shape, f32))

        tSeed = sb("tSeed", [K, 64])
        tWb = sb("tWb", [K + 1, D])
        tMask = sb("tMask", [128, 4 * PPOS])
        tS = sb("tS", [K + 1, PPOS])
        tY = sb("tY", [K, 256])
        tPQ = sb("tPQ", [K, 256])
        tYt = sb("tYt", [128, 4 * K])
        tCh = sb("tCh", [K, 64])
        tOut = sb("tOut", [PPOS, D])
        psPQ = psb("psPQ", [K, 256])
        psE = psb("psE", [K, 256])
        psT = psb("psT", [128, 4 * K])
        psS = psb("psS", [K, PPOS])
        psOa = psb("psOa", [PPOS, D // 2])
        psOb = psb("psOb", [PPOS, D // 2])

        dmaS = nc.alloc_semaphore("dmaS")
        dmaW = nc.alloc_semaphore("dmaW")
        dmaM = nc.alloc_semaphore("dmaM")
        dmaO = nc.alloc_semaphore("dmaO")
        pe = nc.alloc_semaphore("peS")
        dve = nc.alloc_semaphore("dveS")

        # --- input DMAs (issue order = earliest consumer first) ---
        nc.sync.dma_start(out=tSeed[:], in_=dSeed[:]).then_inc(dmaS, 16)
        nc.sync.dma_start(out=tMask[:], in_=dMask[:]).then_inc(dmaM, 16)
        nc.sync.dma_start(out=tWb[:], in_=dWb[0 : K + 1, :]).then_inc(dmaW, 16)
        nc.sync.dma_start(
            out=tS[K : K + 1, :], in_=dWb[K + 1 : K + 2, 0:PPOS]
        ).then_inc(dmaW, 16)

        # --- DVE: seed p into Y ---
        nc.vector.tensor_copy(
            out=tY[:, 0:1], in_=tSeed[:, SEED_P : SEED_P + 1]
        )._wait_ge(dmaS, 16).then_inc(dve, 1)

        # --- scan rounds (PE + DVE interleaved) ---
        # pe ticks: round r (0..6) -> mmP = 2r+1, mmE = 2r+2, so the PQ
        # copy starts while mmE is still streaming.  dve ticks: p-copy = 1,
        # PQ-copy_r = 2r+2, E-copy_r = 2r+3 (last: r=6 -> 14, 15).
        # Y is only built to 128 columns; the second half of the sequence is
        # never materialized in row form (the chunk matmul multiplies by
        # Q128/Q256/Q384 instead).
        cur = tSeed[:, 0:32]
        w = 1
        for r in range(7):
            tQ = cur[:, 0:16]
            tP = cur[:, 16:32]
            c0 = 32 * r
            mq = nc.tensor.matmul(
                psPQ[:, c0 : c0 + 16], lhsT=tP, rhs=tQ, start=True, stop=True
            )
            if r == 0:
                mq._wait_ge(dmaS, 16)
            else:
                mq._wait_ge(dve, 2 * r)
            mp = nc.tensor.matmul(
                psPQ[:, c0 + 16 : c0 + 32], lhsT=tQ, rhs=tP, start=True, stop=True
            ).then_inc(pe, 1)
            if r == 6:
                mp._wait_ge(dmaM, 16)  # absorber for the mask matmuls
            me = nc.tensor.matmul(
                psE[:, w : 2 * w], lhsT=tQ, rhs=tY[:, 0:w], start=True, stop=True
            ).then_inc(pe, 1)
            me._wait_ge(dve, 2 * r + 1)
            nc.vector.tensor_copy(
                out=tPQ[:, c0 : c0 + 32], in_=psPQ[:, c0 : c0 + 32]
            )._wait_ge(pe, 2 * r + 1).then_inc(dve, 1)
            nc.vector.tensor_copy(
                out=tY[:, w : 2 * w], in_=psE[:, w : 2 * w]
            )._wait_ge(pe, 2 * r + 2).then_inc(dve, 1)
            cur = tPQ[:, c0 : c0 + 32]
            w *= 2

        # --- Q256 = Q128 Q128 and Q384 = Q128 Q256 (pe 15, 16) ---
        tQ7 = cur[:, 0:16]   # Q128
        tP7 = cur[:, 16:32]  # P128
        nc.tensor.matmul(
            psPQ[:, 224:240], lhsT=tP7, rhs=tQ7, start=True, stop=True
        )._wait_ge(dve, 14).then_inc(pe, 1)
        nc.vector.tensor_copy(out=tCh[:, 32:48], in_=psPQ[:, 224:240])._wait_ge(
            pe, 15
        ).then_inc(dve, 1)  # dve 16
        nc.vector.tensor_copy(
            out=tCh[:, 0:16], in_=tSeed[:, SEED_I : SEED_I + 16]
        ).then_inc(dve, 1)  # dve 17
        nc.tensor.matmul(
            psPQ[:, 240:256], lhsT=tP7, rhs=tCh[:, 32:48], start=True, stop=True
        )._wait_ge(dve, 16).then_inc(pe, 1)  # pe 16
        nc.vector.tensor_copy(out=tCh[:, 16:32], in_=psPQ[:, 192:208])._wait_ge(
            pe, 16
        ).then_inc(dve, 1)  # dve 18 (after mmQ384: same-bank PE-W/DVE-R rule)
        nc.vector.tensor_copy(out=tCh[:, 48:64], in_=psPQ[:, 240:256]).then_inc(
            dve, 1
        )  # dve 19

        # --- all four transposed chunks in ONE matmul: chunk k rows j hold
        # y_{128k+j}^T = (y_j^T R_k) with rhs = [I | Q128 | Q256 | Q384] ---
        nc.tensor.matmul(
            psT[:, 0:64], lhsT=tY[:, 0:128], rhs=tCh[:, 0:64], start=True, stop=True
        )._wait_ge(dve, 19).then_inc(pe, 1)  # pe 17
        nc.vector.tensor_copy(out=tYt[:], in_=psT[:])._wait_ge(pe, 17).then_inc(dve, 1)

        # --- masked reduction ---
        for kk in range(4):
            m = nc.tensor.matmul(
                psS[:],
                lhsT=tYt[:, kk * K : (kk + 1) * K],
                rhs=tMask[:, kk * PPOS : (kk + 1) * PPOS],
                start=(kk == 0),
                stop=(kk == 3),
            )
            if kk == 0:
                m._wait_ge(dve, 20)
            elif kk == 1:
                m._wait_ge(dmaW, 32)  # absorber for the final matmul below

            if kk == 3:
                m.then_inc(pe, 1)
        nc.vector.tensor_copy(out=tS[0:K, :], in_=psS[:])._wait_ge(pe, 18).then_inc(
            dve, 1
        )

        # --- output projection + store, split in halves so the PSUM copy
        # and output DMA of half 0 overlap the matmul of half 1 ---
        H = D // 2
        nc.tensor.matmul(
            psOa[:], lhsT=tS[:], rhs=tWb[:, 0:H], start=True, stop=True
        )._wait_ge(dve, 21).then_inc(pe, 1)
        nc.tensor.matmul(
            psOb[:], lhsT=tS[:], rhs=tWb[:, H:D], start=True, stop=True
        ).then_inc(pe, 1)
        nc.vector.tensor_copy(out=tOut[:, 0:H], in_=psOa[:])._wait_ge(
            pe, 19
        ).then_inc(dve, 1)
        nc.vector.tensor_copy(out=tOut[:, H:D], in_=psOb[:])._wait_ge(
            pe, 20
        ).then_inc(dve, 1)
        nc.sync.dma_start(out=dOut[:, 0:H], in_=tOut[:, 0:H])._wait_ge(
            dve, 22
        ).then_inc(dmaO, 16)
        nc.sync.dma_start(out=dOut[:, H:D], in_=tOut[:, H:D])._wait_ge(
            dve, 23
        ).then_inc(dmaO, 16)
        nc.sync.wait_ge(dmaO, 32)

    nc.compile()
    return nc


def get_nc():
    key = VARIANT
    if key not in _NC_CACHE:
        if VARIANT == "raw":
            _NC_CACHE[key] = _build_nc_raw()
        else:
            _NC_CACHE[key] = _build_nc(VARIANT)
    return _NC_CACHE[key]


def make_in_maps(pos_initial, pos_transition, W, b):
    T = np.ascontiguousarray(pos_transition, dtype=np.float32)
    seed = np.zeros((K, 64), dtype=np.float32)
    seed[:, SEED_Q1 : SEED_Q1 + 16] = T.T
    seed[:, SEED_P1 : SEED_P1 + 16] = T
    seed[:, SEED_P] = np.asarray(pos_initial, dtype=np.float32).reshape(K)
    seed[:, SEED_I : SEED_I + 16] = np.eye(K, dtype=np.float32)
    wb = np.concatenate(
        [
            W.T.astype(np.float32),
            b.reshape(1, -1).astype(np.float32),
            np.ones((1, D), dtype=np.float32),
        ],
        axis=0,
    )

    j = np.arange(128)[:, None]
    t = np.arange(PPOS)[None, :]
    in_maps = []
    for c in range(NCORES):
        cutoff = (N - 1) - (c * PPOS + t)  # stacked[pos] sums y_j, j <= cutoff
        mask = np.zeros((128, 4 * PPOS), dtype=np.float32)
        for kk in range(4):
            mask[:, kk * PPOS : (kk + 1) * PPOS] = (j + 128 * kk <= cutoff).astype(
                np.float32
            )
        in_maps.append(
            {"seed": seed, "wb": np.ascontiguousarray(wb), "mask": mask}
        )
    return in_maps


def assemble_output(per_core_results):
    if VARIANT in ("f32", "raw"):
        return np.concatenate(
            [np.asarray(per_core_results[c]["out"]) for c in range(NCORES)], axis=0
        )
    out = np.empty((N, D), dtype=np.float32)
    for c in range(NCORES):
        arr = np.asarray(per_core_results[c]["out"])  # [128, 4*PPOS]
        for e in range(4):
            out[c * PPOS : (c + 1) * PPOS, e * 128 : (e + 1) * 128] = arr[
                :, e * PPOS : (e + 1) * PPOS
            ].T
    return out


def kernel(**inputs):
    pos_initial = np.asarray(inputs["pos_initial"], dtype=np.float32)
    pos_transition = np.asarray(inputs["pos_transition"], dtype=np.float32)
    W = np.asarray(inputs["W"], dtype=np.float32)
    b = np.asarray(inputs["b"], dtype=np.float32)
    n = int(inputs["sentence_len"])

    if n != N or pos_initial.shape[0] != K or W.shape != (D, K):
        return _host_fallback(pos_initial, pos_transition, W, b, n)

    from concourse.bass_utils import run_bass_kernel_spmd

    nc = get_nc()
    in_maps = make_in_maps(pos_initial, pos_transition, W, b)
    kwargs = {"trace": True} if TRACE else {}
    res = run_bass_kernel_spmd(nc, in_maps, core_ids=list(range(NCORES)), **kwargs)
    global LAST_RESULT
    LAST_RESULT = res
    return assemble_output(res.results)


if __name__ == "__main__":
    rng = np.random.default_rng(0)
    p = rng.normal(size=(K, 1)).astype(np.float32)
    A = rng.normal(size=(K, K)).astype(np.float32)
    q, r = np.linalg.qr(A)
    T = (q * np.sign(np.diag(r))[None, :]).astype(np.float32)
    W = rng.uniform(-0.25, 0.25, size=(D, K)).astype(np.float32)
    b = rng.uniform(-0.25, 0.25, size=(D,)).astype(np.float32)
    ref = _host_fallback(p, T, W, b, N)
    act = kernel(pos_initial=p, pos_transition=T, W=W, b=b, sentence_len=N)
    err = np.abs(act - ref).max() / np.abs(ref).max()
    print("max rel err vs host closed form:", err)



# revision 6
# speedup vs baseline: 1.0555x; 1.0555x over previous
"""Trainium2 Bass kernel for nn_AutomatonPELayer (n=512, k=16, d=512).

Math: the reference solves B x = tile(p) with B = I - kron(shift, T),
which is block upper-bidiagonal => stacked[i] = s_{n-1-i} where
s_m = sum_{j<=m} T^j p.  In homogeneous coordinates s-hat_m = [s_m; 1],
the prefix satisfies s-hat_{w+m} = M_w s-hat_m with
M_w = [[T^w, s_{w-1}], [0, 1]], and M_a M_b = M_{a+b}.  So a log-depth
doubling scan on the 17x17 M (tracking both M and Q = M^T, since the PE
computes lhsT.T @ rhs) builds S64 = [s-hat_0 .. s-hat_63] in 6 rounds.
Core with jump q then applies M_{64q} = M_256^bb * M_{64 ba} (q = ba+4bb)
as two data-selected matmuls: the selector matrices are 0/1 masks sent
from the host (layout-only), applied with copy_predicated onto
identity-prefilled tiles, so all 8 cores run one instruction stream.
The projection pe = C^T Wb is done transposed (4 matmuls with 128
output partitions: psO[:, 64e:64e+64] = Wb_e^T C), and the homogeneous
ones-row provides the bias term for free.

Host work is layout-only: building M1/Q1 from T and p, identity /
0-1 mask tiles, W^T|b concat, and transpose+flip on output assembly.

Hardware notes:
  - ONE input DMA and ONE output DMA per core (each dma_start costs
    ~700ns of serial Sync-engine descriptor generation).
  - Each scan round: PE does mmQ, mmM (pair) then mmS; the [Q|M] pair
    lands in adjacent PSUM columns so ONE DVE copy moves both, while
    the ACT engine copies the S extension in parallel.
  - ONE worker semaphore (qm) counts every DVE/ACT copy and blend; its
    wait thresholds are chosen so each consumer's single wait covers all
    of its producers (cumulative counts, order-independent).
  - PSUM columns are never recycled, so no WAR waits.

qm increments (cumulative):
  rounds r=1..6: cpQM_r + cpS_r          -> 2r   (12 after round 6)
  cpQM7 -> 13 | bA1 -> 14 | bA2 -> 15 | cpQ89 -> 16 | bA3 -> 17
  bB -> 18 | cpCa -> 19 | cpCb -> 20 | cpO1 -> 21 | cpO2 -> 22
pe increments: mmQ_r=3r-2 mmM_r=3r-1 mmS_r=3r (r=1..6), mmQ7=19,
  mmM7=20, mm8a=21, mm8b=22, mmA=23, mmB=24, mmP0..3=25..28.
"""

import numpy as np

N = 512  # sentence length handled by the device kernel
K = 16  # num states
H = K + 1  # homogeneous dim
D = 512  # embed dim
NCORES = 8
PPOS = N // NCORES  # positions per core (64)

# tAll column map
C_PAIR0 = 0  # [Q1 | M1]
C_JAT = 34
C_JBT = 51
C_BA1 = 68
C_BA2 = 85
C_BA3 = 102
C_BB = 119
C_WB = 136  # [17, 512]
C_S = 648  # s-hat_0 at col 648, S grows to col 712
C_PAIRS = 712  # pair_r (r=1..7) at 712+34(r-1), pair8 = [Q192|Q256] at 950
NCOL_IN = 649
NCOL_ALL = 984

_NC_CACHE = {}

VARIANT = "raw"

# Set by an external harness to capture a profile; grading path leaves these.
TRACE = False
LAST_RESULT = None


def _host_fallback(p, T, W, b, n):
    # Closed-form reference for shapes the compiled kernel doesn't handle.
    p = p.reshape(-1).astype(np.float64)
    T = T.astype(np.float64)
    k = p.shape[0]
    stacked = np.zeros((n, k), dtype=np.float64)
    acc = np.zeros(k, dtype=np.float64)
    for i in range(n - 1, -1, -1):
        acc = p + (T @ acc if i < n - 1 else 0.0)
        stacked[i] = acc
    pe = stacked @ W.astype(np.float64).T + b.astype(np.float64)
    return pe.astype(np.float32)


def _build_nc_raw():
    """Hand-scheduled Bacc build: no TileContext, explicit semaphores.

    Engine streams (each instruction carries at most one wait):
      SP  : dma in | dma out | wait out
      PE  : 6 rounds of (mmQ, mmM, mmS) | mmQ7, mmM7 | mm8a, mm8b |
            mmA | mmB | mmP0..3
      DVE : cpQM 1..7 | bA1 bA2 | cpQ89 | bA3 bB | cpCb | cpO1
      ACT : cpS 1..6 | cpCa | cpO2
    """
    from contextlib import ExitStack

    import concourse.mybir as mybir
    from concourse import bacc

    f32 = mybir.dt.float32
    nc = bacc.Bacc("TRN2", target_bir_lowering=False)

    dIn = nc.dram_tensor("inp", [H, NCOL_IN], f32, kind="ExternalInput")
    dOut = nc.dram_tensor("out", [128, 4 * PPOS], f32, kind="ExternalOutput")

    with ExitStack() as ctx:
        def sb(name, shape):
            return ctx.enter_context(nc.sbuf_tensor(name, shape, f32))

        def psb(name, shape):
            return ctx.enter_context(nc.psum_tensor(name, shape, f32))

        tAll = sb("tAll", [H, NCOL_ALL])
        tCa = sb("tCa", [H, PPOS])
        tCb = sb("tCb", [H, PPOS])
        tOut = sb("tOut", [128, 4 * PPOS])
        psQM = psb("psQM", [H, 34 * 7])
        psQ89 = psb("psQ89", [H, 34])
        psS = psb("psS", [H, 63])
        psCa = psb("psCa", [H, PPOS])
        psCb = psb("psCb", [H, PPOS])
        # two PSUM tensors so the DVE and ACT output copies never read the
        # same PSUM bank concurrently (observed to fail on hardware)
        psOa = psb("psOa", [128, 2 * PPOS])
        psOb = psb("psOb", [128, 2 * PPOS])

        dmaIn = nc.alloc_semaphore("dmaIn")
        dmaO = nc.alloc_semaphore("dmaO")
        pe = nc.alloc_semaphore("peS")
        qm = nc.alloc_semaphore("qmS")  # all worker (DVE/ACT) copies + blends

        def pair(r):
            # [Q_{2^r} | M_{2^r}] columns in tAll
            if r == 0:
                return tAll[:, C_PAIR0 : C_PAIR0 + 34]
            return tAll[:, C_PAIRS + 34 * (r - 1) : C_PAIRS + 34 * r]

        # --- input DMA ---
        nc.sync.dma_start(out=tAll[:, 0:NCOL_IN], in_=dIn[:]).then_inc(dmaIn, 16)

        # --- scan rounds r=1..6 (w = 2^(r-1)) ---
        for r in range(1, 7):
            w = 1 << (r - 1)
            prev = pair(r - 1)
            tQ = prev[:, 0:17]
            tM = prev[:, 17:34]
            po = 34 * (r - 1)
            soff = w - 1
            mq = nc.tensor.matmul(
                psQM[:, po : po + 17], lhsT=tM, rhs=tQ, start=True, stop=True
            ).then_inc(pe, 1)
            mm = nc.tensor.matmul(
                psQM[:, po + 17 : po + 34], lhsT=tQ, rhs=tM, start=True, stop=True
            ).then_inc(pe, 1)
            ms = nc.tensor.matmul(
                psS[:, soff : soff + w],
                lhsT=tQ,
                rhs=tAll[:, C_S : C_S + w],
                start=True,
                stop=True,
            ).then_inc(pe, 1)
            if r == 1:
                mq._wait_ge(dmaIn, 16)
                mm._wait_ge(dmaIn, 16)
                ms._wait_ge(dmaIn, 16)
            else:
                mq._wait_ge(qm, 2 * (r - 1))
                mm._wait_ge(qm, 2 * (r - 1))
                ms._wait_ge(qm, 2 * (r - 1))
            nc.vector.tensor_copy(out=pair(r)[:], in_=psQM[:, po : po + 34])._wait_ge(
                pe, 3 * r - 1
            ).then_inc(qm, 1)
            nc.scalar.copy(
                out=tAll[:, C_S + w : C_S + 2 * w], in_=psS[:, soff : soff + w]
            )._wait_ge(pe, 3 * r).then_inc(qm, 1)

        # --- r7: [Q128 | M128] (no S extension); pe 19, 20 ---
        p6 = pair(6)
        nc.tensor.matmul(
            psQM[:, 204:221], lhsT=p6[:, 17:34], rhs=p6[:, 0:17], start=True, stop=True
        )._wait_ge(qm, 12).then_inc(pe, 1)
        nc.tensor.matmul(
            psQM[:, 221:238], lhsT=p6[:, 0:17], rhs=p6[:, 17:34], start=True, stop=True
        )._wait_ge(qm, 12).then_inc(pe, 1)
        nc.vector.tensor_copy(out=pair(7)[:], in_=psQM[:, 204:238])._wait_ge(
            pe, 20
        ).then_inc(qm, 1)  # qm 13

        # --- r8: Q192 = Q128 Q64, Q256 = Q128 Q128; pe 21, 22 ---
        p7 = pair(7)
        nc.tensor.matmul(
            psQ89[:, 0:17], lhsT=p7[:, 17:34], rhs=p6[:, 0:17], start=True, stop=True
        )._wait_ge(qm, 13).then_inc(pe, 1)
        nc.tensor.matmul(
            psQ89[:, 17:34], lhsT=p7[:, 17:34], rhs=p7[:, 0:17], start=True, stop=True
        )._wait_ge(qm, 13).then_inc(pe, 1)

        # --- DVE: selector blends (interleaved with cpQ89) ---
        tJaT = tAll[:, C_JAT : C_JAT + 17]
        tJbT = tAll[:, C_JBT : C_JBT + 17]
        nc.vector.copy_predicated(
            out=tJaT,
            mask=tAll[:, C_BA1 : C_BA1 + 17].bitcast(mybir.dt.uint32),
            data=p6[:, 0:17],
        )._wait_ge(qm, 12).then_inc(qm, 1)  # qm 14 (Q64)
        nc.vector.copy_predicated(
            out=tJaT,
            mask=tAll[:, C_BA2 : C_BA2 + 17].bitcast(mybir.dt.uint32),
            data=p7[:, 0:17],
        )._wait_ge(qm, 14).then_inc(qm, 1)  # qm 15 (Q128)
        nc.vector.tensor_copy(out=pair(8)[:], in_=psQ89[:])._wait_ge(pe, 22).then_inc(
            qm, 1
        )  # qm 16
        p8 = pair(8)
        nc.vector.copy_predicated(
            out=tJaT,
            mask=tAll[:, C_BA3 : C_BA3 + 17].bitcast(mybir.dt.uint32),
            data=p8[:, 0:17],
        )._wait_ge(qm, 16).then_inc(qm, 1)  # qm 17 (Q192)
        nc.vector.copy_predicated(
            out=tJbT,
            mask=tAll[:, C_BB : C_BB + 17].bitcast(mybir.dt.uint32),
            data=p8[:, 17:34],
        )._wait_ge(qm, 16).then_inc(qm, 1)  # qm 18 (Q256)

        # --- applies: Ca = Ja S64, Cb = Jb Ca; pe 23, 24 ---
        nc.tensor.matmul(
            psCa[:], lhsT=tJaT, rhs=tAll[:, C_S : C_S + PPOS], start=True, stop=True
        )._wait_ge(qm, 17).then_inc(pe, 1)
        nc.scalar.copy(out=tCa[:], in_=psCa[:])._wait_ge(pe, 23).then_inc(qm, 1)  # 19
        nc.tensor.matmul(
            psCb[:], lhsT=tJbT, rhs=tCa[:], start=True, stop=True
        )._wait_ge(qm, 19).then_inc(pe, 1)
        nc.vector.tensor_copy(out=tCb[:], in_=psCb[:])._wait_ge(pe, 24).then_inc(
            qm, 1
        )  # qm 20

        # --- projection: psO[:, 64e:64e+64] = Wb_e^T Cb; pe 25..28 ---
        for e in range(4):
            dst = psOa if e < 2 else psOb
            nc.tensor.matmul(
                dst[:, PPOS * (e % 2) : PPOS * (e % 2 + 1)],
                lhsT=tAll[:, C_WB + 128 * e : C_WB + 128 * (e + 1)],
                rhs=tCb[:],
                start=True,
                stop=True,
            )._wait_ge(qm, 20).then_inc(pe, 1)

        # --- output copy (split DVE/ACT) + single DMA ---
        nc.vector.tensor_copy(out=tOut[:, 0:128], in_=psOa[:])._wait_ge(
            pe, 26
        ).then_inc(qm, 1)  # qm 21
        nc.scalar.copy(out=tOut[:, 128:256], in_=psOb[:])._wait_ge(
            pe, 28
        ).then_inc(qm, 1)  # qm 22
        nc.sync.dma_start(out=dOut[:], in_=tOut[:])._wait_ge(qm, 22).then_inc(
            dmaO, 16
        )
        nc.sync.wait_ge(dmaO, 16)

    nc.compile()
    return nc


def get_nc():
    key = VARIANT
    if key not in _NC_CACHE:
        _NC_CACHE[key] = _build_nc_raw()
    return _NC_CACHE[key]


def make_in_maps(pos_initial, pos_transition, W, b):
    T = np.ascontiguousarray(pos_transition, dtype=np.float32)
    p = np.asarray(pos_initial, dtype=np.float32).reshape(K)

    M1 = np.zeros((H, H), dtype=np.float32)
    M1[0:K, 0:K] = T
    M1[0:K, K] = p
    M1[K, K] = 1.0
    I17 = np.eye(H, dtype=np.float32)
    ones = np.ones((H, H), dtype=np.float32)
    zeros = np.zeros((H, H), dtype=np.float32)
    wb = np.concatenate(
        [W.T.astype(np.float32), b.reshape(1, -1).astype(np.float32)], axis=0
    )
    s0 = np.concatenate([p, [1.0]]).astype(np.float32)

    in_maps = []
    for c in range(NCORES):
        q = (NCORES - 1) - c
        ba, bb = q % 4, q // 4
        inp = np.zeros((H, NCOL_IN), dtype=np.float32)
        inp[:, 0:17] = M1.T
        inp[:, 17:34] = M1
        inp[:, C_JAT : C_JAT + 17] = I17
        inp[:, C_JBT : C_JBT + 17] = I17
        inp[:, C_BA1 : C_BA1 + 17] = ones if ba == 1 else zeros
        inp[:, C_BA2 : C_BA2 + 17] = ones if ba == 2 else zeros
        inp[:, C_BA3 : C_BA3 + 17] = ones if ba == 3 else zeros
        inp[:, C_BB : C_BB + 17] = ones if bb == 1 else zeros
        inp[:, C_WB : C_WB + D] = wb
        inp[:, C_S] = s0
        in_maps.append({"inp": np.ascontiguousarray(inp)})
    return in_maps


def assemble_output(per_core_results):
    out = np.empty((N, D), dtype=np.float32)
    for c in range(NCORES):
        arr = np.asarray(per_core_results[c]["out"])  # [128, 256]
        for e in range(4):
            blk = arr[:, PPOS * e : PPOS * (e + 1)]  # [128, 64]
            out[PPOS * c : PPOS * (c + 1), 128 * e : 128 * (e + 1)] = blk[:, ::-1].T
    return out


def kernel(**inputs):
    pos_initial = np.asarray(inputs["pos_initial"], dtype=np.float32)
    pos_transition = np.asarray(inputs["pos_transition"], dtype=np.float32)
    W = np.asarray(inputs["W"], dtype=np.float32)
    b = np.asarray(inputs["b"], dtype=np.float32)
    n = int(inputs["sentence_len"])

    if n != N or pos_initial.shape[0] != K or W.shape != (D, K):
        return _host_fallback(pos_initial, pos_transition, W, b, n)

    from concourse.bass_utils import run_bass_kernel_spmd

    nc = get_nc()
    in_maps = make_in_maps(pos_initial, pos_transition, W, b)
    kwargs = {"trace": True} if TRACE else {}
    res = run_bass_kernel_spmd(nc, in_maps, core_ids=list(range(NCORES)), **kwargs)
    global LAST_RESULT
    LAST_RESULT = res
    return assemble_output(res.results)


if __name__ == "__main__":
    rng = np.random.default_rng(0)
    p = rng.normal(size=(K, 1)).astype(np.float32)
    A = rng.normal(size=(K, K)).astype(np.float32)
    q, r = np.linalg.qr(A)
    T = (q * np.sign(np.diag(r))[None, :]).astype(np.float32)
    W = rng.uniform(-0.25, 0.25, size=(D, K)).astype(np.float32)
    b = rng.uniform(-0.25, 0.25, size=(D,)).astype(np.float32)
    ref = _host_fallback(p, T, W, b, N)
    act = kernel(pos_initial=p, pos_transition=T, W=W, b=b, sentence_len=N)
    err = np.abs(act - ref).max() / np.abs(ref).max()
    print("max rel err vs host closed form:", err)


# revision 11
# speedup vs baseline: 1.2149x; 1.1510x over previous
"""Trainium2 Bass kernel for nn_AutomatonPELayer (n=512, k=16, d=512).

Math: the reference solves B x = tile(p) with B = I - kron(shift, T),
which is block upper-bidiagonal => stacked[i] = s_{n-1-i} where
s_m = sum_{j<=m} T^j p.  In homogeneous coordinates s-hat_m = [s_m; 1],
the prefix satisfies s-hat_{w+m} = M_w s-hat_m with
M_w = [[T^w, s_{w-1}], [0, 1]], and M_a M_b = M_{a+b}.  So a log-depth
doubling scan on the 17x17 M (tracking both M and Q = M^T, since the PE
computes lhsT.T @ rhs) builds S64 = [s-hat_0 .. s-hat_63] in 6 rounds.
Core with jump q then applies M_{64q} = M_256^bb * M_{64 ba} (q = ba+4bb)
as two data-selected matmuls: the selector matrices are 0/1 masks sent
from the host (layout-only), applied with copy_predicated onto
identity-prefilled tiles, so all 8 cores run one instruction stream.
The projection pe-block = Cb^T Wb is two float32r matmuls with 256-wide
moving dim (1 cycle/row vs fp32's 4); the homogeneous ones-row provides
the bias for free.  Host work is layout-only: M1/Q1 assembly, identity /
0-1 mask tiles, W^T|b concat, row-reversal on output assembly.

Hardware notes (from trace analysis of earlier revisions):
  - DMA rows must be <= 2048B or the transfer serializes on one queue
    (~100ns/descriptor); inputs are split into two DMAs of 548B/2048B
    rows.  Each dma_start also costs ~700ns of Sync-engine descriptor
    generation, so there are exactly two input DMAs and one output DMA.
  - fp32 matmuls run LOW+HIGH double passes (4 cyc/row); float32r with
    moving dim >= 256 runs single pass.  The scan and applies stay fp32
    for precision; the projection is float32r.  fp32r operands must come
    from fp32r-typed producers (BIR verifier), so Wb lands in an f32r
    tile via its own DMA and Cb is rounded into an f32r tile by the DVE
    copy; the zero padding of Cb to 128 columns is a DVE memset.
  - The per-round pair copy (DVE, [Q|M] adjacent in PSUM) gates the next
    round; the S-extension copy (ACT) is off the critical path with its
    own semaphore.  Two engines never read the same PSUM tensor
    concurrently (observed hardware failure).
  - PSUM columns are never recycled, so no WAR waits.
"""

import numpy as np

N = 512  # sentence length handled by the device kernel
K = 16  # num states
H = K + 1  # homogeneous dim
D = 512  # embed dim
NCORES = 8
PPOS = N // NCORES  # positions per core (64)

# tAll column map
C_PAIR0 = 0  # [Q1 | M1]
C_JAT = 34
C_JBT = 51
C_BA1 = 68
C_BA2 = 85
C_BA3 = 102
C_BB = 119
C_S = 136  # s-hat_0 at col 136, S grows to col 200
C_PAIRS = 200  # pair_r (r=1..7) at 200+34(r-1), pair8 at 438
NCOL_IN1 = 137  # first input DMA: cols 0:137 (548 B/row)
NCOL_ALL = 472

_NC_CACHE = {}

VARIANT = "raw"

# Set by an external harness to capture a profile; grading path leaves these.
TRACE = False
LAST_RESULT = None


def _host_fallback(p, T, W, b, n):
    # Closed-form reference for shapes the compiled kernel doesn't handle.
    p = p.reshape(-1).astype(np.float64)
    T = T.astype(np.float64)
    k = p.shape[0]
    stacked = np.zeros((n, k), dtype=np.float64)
    acc = np.zeros(k, dtype=np.float64)
    for i in range(n - 1, -1, -1):
        acc = p + (T @ acc if i < n - 1 else 0.0)
        stacked[i] = acc
    pe = stacked @ W.astype(np.float64).T + b.astype(np.float64)
    return pe.astype(np.float32)


def _build_nc_raw():
    """Hand-scheduled Bacc build: no TileContext, explicit semaphores.

    Engine streams:
      SP  : dma in1 | dma in2 | dma out | wait out
      PE  : 6 rounds of (mmQ, mmM, mmS) | mmQ7, mmM7 | mm8a, mm8b |
            mmA | mmB | mmP0, mmP1
      DVE : memset CbPad | cpQM 1..7 | bA1 bA2 | cpQ89 | bA3 bB |
            cpCa | cpCb | cpO1
      ACT : cpS 1..6 | cpO2
    """
    from contextlib import ExitStack

    import concourse.mybir as mybir
    from concourse import bacc

    f32 = mybir.dt.float32
    f32r = mybir.dt.float32r
    nc = bacc.Bacc("TRN2", target_bir_lowering=False)

    dIn1 = nc.dram_tensor("inp1", [H, NCOL_IN1], f32, kind="ExternalInput")
    dIn2 = nc.dram_tensor("inp2", [H, D], f32r, kind="ExternalInput")
    dOut = nc.dram_tensor("out", [PPOS, D], f32, kind="ExternalOutput")

    with ExitStack() as ctx:
        tAll = ctx.enter_context(nc.sbuf_tensor("tAll", [H, NCOL_ALL], f32))
        tWb = ctx.enter_context(nc.sbuf_tensor("tWb", [H, D], f32r))
        tCa = ctx.enter_context(nc.sbuf_tensor("tCa", [H, 128], f32))
        tCbP = ctx.enter_context(nc.sbuf_tensor("tCbP", [H, 128], f32r))
        tOut = ctx.enter_context(nc.sbuf_tensor("tOut", [PPOS, D], f32))

        def psb(name, shape):
            return ctx.enter_context(nc.psum_tensor(name, shape, f32))

        psQM = psb("psQM", [H, 34 * 7])
        psQ89 = psb("psQ89", [H, 34])
        psS = psb("psS", [H, 63])
        psCa = psb("psCa", [H, PPOS])
        psCb = psb("psCb", [H, 128])
        psOa = psb("psOa", [128, 256])
        psOb = psb("psOb", [128, 256])

        dmaIn = nc.alloc_semaphore("dmaIn")
        dmaIn2 = nc.alloc_semaphore("dmaIn2")
        dmaO = nc.alloc_semaphore("dmaO")
        pe = nc.alloc_semaphore("peS")
        qmP = nc.alloc_semaphore("qmP")  # DVE stream
        qmS = nc.alloc_semaphore("qmS")  # ACT scan copies
        outS = nc.alloc_semaphore("outS")

        npe = [0]  # pe count after each PE instruction
        nqp = [0]  # qmP (DVE) count
        nqs = [0]  # qmS (ACT) count

        def pe_inc(instr):
            npe[0] += 1
            return instr.then_inc(pe, 1)

        def qp_inc(instr):
            nqp[0] += 1
            return instr.then_inc(qmP, 1)

        def qs_inc(instr):
            nqs[0] += 1
            return instr.then_inc(qmS, 1)

        def pair(r):
            # [Q_{2^r} | M_{2^r}] columns in tAll
            if r == 0:
                return tAll[:, C_PAIR0 : C_PAIR0 + 34]
            return tAll[:, C_PAIRS + 34 * (r - 1) : C_PAIRS + 34 * r]

        # --- input DMAs (seed first; Wb only needed at the projection) ---
        nc.sync.dma_start(out=tAll[:, 0:NCOL_IN1], in_=dIn1[:]).then_inc(dmaIn, 16)
        nc.sync.dma_start(out=tWb[:], in_=dIn2[:]).then_inc(dmaIn2, 16)

        # --- DVE: zero the fp32 Ca padding columns once, up front; the
        # apply-B matmul then writes zeros into psCb cols 64:128, and the
        # rounding copy fills all 128 fp32r Cb columns ---
        qp_inc(nc.vector.memset(tCa[:, PPOS:128], 0.0))

        # --- scan rounds r=1..6 (w = 2^(r-1)) ---
        cpq_at = {}  # round -> qmP count of its pair copy
        cps_at = {}  # round -> qmS count of its S copy
        mm_at = {}  # tag -> pe count
        for r in range(1, 7):
            w = 1 << (r - 1)
            prev = pair(r - 1)
            tQ = prev[:, 0:17]
            tM = prev[:, 17:34]
            po = 34 * (r - 1)
            soff = w - 1
            mq = pe_inc(
                nc.tensor.matmul(
                    psQM[:, po : po + 17], lhsT=tM, rhs=tQ, start=True, stop=True
                )
            )
            mm = pe_inc(
                nc.tensor.matmul(
                    psQM[:, po + 17 : po + 34], lhsT=tQ, rhs=tM, start=True, stop=True
                )
            )
            mm_at[("m", r)] = npe[0]
            ms = pe_inc(
                nc.tensor.matmul(
                    psS[:, soff : soff + w],
                    lhsT=tQ,
                    rhs=tAll[:, C_S : C_S + w],
                    start=True,
                    stop=True,
                )
            )
            mm_at[("s", r)] = npe[0]
            if r == 1:
                mq._wait_ge(dmaIn, 16)
                mm._wait_ge(dmaIn, 16)
                ms._wait_ge(dmaIn, 16)
            else:
                mq._wait_ge(qmP, cpq_at[r - 1])
                mm._wait_ge(qmP, cpq_at[r - 1])
                ms._wait_ge(qmP, cpq_at[r - 1])
                # second wait -> EVSEM split on the PE sequencer
                ms.wait_op(qmS, cps_at[r - 1], "sem-ge", check=False)
            qp_inc(
                nc.vector.tensor_copy(
                    out=pair(r)[:], in_=psQM[:, po : po + 34]
                )._wait_ge(pe, mm_at[("m", r)])
            )
            cpq_at[r] = nqp[0]
            qs_inc(
                nc.scalar.copy(
                    out=tAll[:, C_S + w : C_S + 2 * w], in_=psS[:, soff : soff + w]
                )._wait_ge(pe, mm_at[("s", r)])
            )
            cps_at[r] = nqs[0]

        # --- r7: [Q128 | M128] (no S extension) ---
        p6 = pair(6)
        pe_inc(
            nc.tensor.matmul(
                psQM[:, 204:221], lhsT=p6[:, 17:34], rhs=p6[:, 0:17],
                start=True, stop=True,
            )._wait_ge(qmP, cpq_at[6])
        )
        pe_inc(
            nc.tensor.matmul(
                psQM[:, 221:238], lhsT=p6[:, 0:17], rhs=p6[:, 17:34],
                start=True, stop=True,
            )._wait_ge(qmP, cpq_at[6])
        )
        mm_at["r7"] = npe[0]
        qp_inc(
            nc.vector.tensor_copy(out=pair(7)[:], in_=psQM[:, 204:238])._wait_ge(
                pe, mm_at["r7"]
            )
        )
        cpq7 = nqp[0]

        # --- r8: Q192 = Q128 Q64, Q256 = Q128 Q128 ---
        p7 = pair(7)
        pe_inc(
            nc.tensor.matmul(
                psQ89[:, 0:17], lhsT=p7[:, 17:34], rhs=p6[:, 0:17],
                start=True, stop=True,
            )._wait_ge(qmP, cpq7)
        )
        pe_inc(
            nc.tensor.matmul(
                psQ89[:, 17:34], lhsT=p7[:, 17:34], rhs=p7[:, 0:17],
                start=True, stop=True,
            )._wait_ge(qmP, cpq7)
        )
        mm_at["r8"] = npe[0]

        # --- DVE: selector blends (interleaved with cpQ89) ---
        tJaT = tAll[:, C_JAT : C_JAT + 17]
        tJbT = tAll[:, C_JBT : C_JBT + 17]
        qp_inc(
            nc.vector.copy_predicated(
                out=tJaT,
                mask=tAll[:, C_BA1 : C_BA1 + 17].bitcast(mybir.dt.uint32),
                data=p6[:, 0:17],
            )._wait_ge(qmP, cpq7)
        )
        bA1 = nqp[0]
        qp_inc(
            nc.vector.copy_predicated(
                out=tJaT,
                mask=tAll[:, C_BA2 : C_BA2 + 17].bitcast(mybir.dt.uint32),
                data=p7[:, 0:17],
            )._wait_ge(qmP, bA1)
        )
        qp_inc(
            nc.vector.tensor_copy(out=pair(8)[:], in_=psQ89[:])._wait_ge(
                pe, mm_at["r8"]
            )
        )
        cpq89 = nqp[0]
        p8 = pair(8)
        qp_inc(
            nc.vector.copy_predicated(
                out=tJaT,
                mask=tAll[:, C_BA3 : C_BA3 + 17].bitcast(mybir.dt.uint32),
                data=p8[:, 0:17],
            )._wait_ge(qmP, cpq89)
        )
        bA3 = nqp[0]
        qp_inc(
            nc.vector.copy_predicated(
                out=tJbT,
                mask=tAll[:, C_BB : C_BB + 17].bitcast(mybir.dt.uint32),
                data=p8[:, 17:34],
            )._wait_ge(qmP, cpq89)
        )

        # --- applies (fp32): Ca = Ja S64, Cb = Jb Ca ---
        ma = pe_inc(
            nc.tensor.matmul(
                psCa[:], lhsT=tJaT, rhs=tAll[:, C_S : C_S + PPOS],
                start=True, stop=True,
            )
        )
        ma._wait_ge(qmP, bA3)
        ma.wait_op(qmS, cps_at[6], "sem-ge", check=False)  # S64 complete
        mm_at["ma"] = npe[0]
        qp_inc(
            nc.vector.tensor_copy(out=tCa[:, 0:PPOS], in_=psCa[:])._wait_ge(
                pe, mm_at["ma"]
            )
        )
        cpCa = nqp[0]
        mb = pe_inc(
            nc.tensor.matmul(psCb[:], lhsT=tJbT, rhs=tCa[:], start=True, stop=True)
        )
        mb._wait_ge(qmP, cpCa)  # also covers bB < cpCa
        mm_at["mb"] = npe[0]
        # DVE copy rounds Cb (incl. zero padding) into the fp32r operand
        qp_inc(
            nc.vector.tensor_copy(out=tCbP[:], in_=psCb[:])._wait_ge(
                pe, mm_at["mb"]
            )
        )
        cpCb = nqp[0]

        # --- projection: psO = CbPad^T Wb (float32r, 256-wide moving) ---
        mp0 = pe_inc(
            nc.tensor.matmul(
                psOa[:], lhsT=tCbP[:], rhs=tWb[:, 0:256], start=True, stop=True
            )
        )
        mp0._wait_ge(qmP, cpCb)
        mp0.wait_op(dmaIn2, 16, "sem-ge", check=False)  # Wb present
        mm_at["p0"] = npe[0]
        pe_inc(
            nc.tensor.matmul(
                psOb[:], lhsT=tCbP[:], rhs=tWb[:, 256:512], start=True, stop=True
            )._wait_ge(qmP, cpCb)
        )
        mm_at["p1"] = npe[0]

        # --- output copies (separate PSUM tensors per engine) + DMA ---
        nc.vector.tensor_copy(out=tOut[:, 0:256], in_=psOa[0:PPOS, :])._wait_ge(
            pe, mm_at["p0"]
        ).then_inc(outS, 1)
        nc.scalar.copy(out=tOut[:, 256:512], in_=psOb[0:PPOS, :])._wait_ge(
            pe, mm_at["p1"]
        ).then_inc(outS, 1)
        nc.sync.dma_start(out=dOut[:], in_=tOut[:])._wait_ge(outS, 2).then_inc(
            dmaO, 16
        )
        nc.sync.wait_ge(dmaO, 16)

    nc.compile()
    return nc


def get_nc():
    key = VARIANT
    if key not in _NC_CACHE:
        _NC_CACHE[key] = _build_nc_raw()
    return _NC_CACHE[key]


def make_in_maps(pos_initial, pos_transition, W, b):
    T = np.ascontiguousarray(pos_transition, dtype=np.float32)
    p = np.asarray(pos_initial, dtype=np.float32).reshape(K)

    M1 = np.zeros((H, H), dtype=np.float32)
    M1[0:K, 0:K] = T
    M1[0:K, K] = p
    M1[K, K] = 1.0
    I17 = np.eye(H, dtype=np.float32)
    ones = np.ones((H, H), dtype=np.float32)
    zeros = np.zeros((H, H), dtype=np.float32)
    wb = np.concatenate(
        [W.T.astype(np.float32), b.reshape(1, -1).astype(np.float32)], axis=0
    )
    s0 = np.concatenate([p, [1.0]]).astype(np.float32)

    in_maps = []
    for c in range(NCORES):
        q = (NCORES - 1) - c
        ba, bb = q % 4, q // 4
        inp = np.zeros((H, NCOL_IN1), dtype=np.float32)
        inp[:, 0:17] = M1.T
        inp[:, 17:34] = M1
        inp[:, C_JAT : C_JAT + 17] = I17
        inp[:, C_JBT : C_JBT + 17] = I17
        inp[:, C_BA1 : C_BA1 + 17] = ones if ba == 1 else zeros
        inp[:, C_BA2 : C_BA2 + 17] = ones if ba == 2 else zeros
        inp[:, C_BA3 : C_BA3 + 17] = ones if ba == 3 else zeros
        inp[:, C_BB : C_BB + 17] = ones if bb == 1 else zeros
        inp[:, C_S] = s0
        in_maps.append(
            {"inp1": np.ascontiguousarray(inp), "inp2": np.ascontiguousarray(wb)}
        )
    return in_maps


def assemble_output(per_core_results):
    out = np.empty((N, D), dtype=np.float32)
    for c in range(NCORES):
        arr = np.asarray(per_core_results[c]["out"])  # [64, 512]
        out[PPOS * c : PPOS * (c + 1), :] = arr[::-1, :]
    return out


def kernel(**inputs):
    pos_initial = np.asarray(inputs["pos_initial"], dtype=np.float32)
    pos_transition = np.asarray(inputs["pos_transition"], dtype=np.float32)
    W = np.asarray(inputs["W"], dtype=np.float32)
    b = np.asarray(inputs["b"], dtype=np.float32)
    n = int(inputs["sentence_len"])

    if n != N or pos_initial.shape[0] != K or W.shape != (D, K):
        return _host_fallback(pos_initial, pos_transition, W, b, n)

    from concourse.bass_utils import run_bass_kernel_spmd

    nc = get_nc()
    in_maps = make_in_maps(pos_initial, pos_transition, W, b)
    kwargs = {"trace": True} if TRACE else {}
    res = run_bass_kernel_spmd(nc, in_maps, core_ids=list(range(NCORES)), **kwargs)
    global LAST_RESULT
    LAST_RESULT = res
    return assemble_output(res.results)


if __name__ == "__main__":
    rng = np.random.default_rng(0)
    p = rng.normal(size=(K, 1)).astype(np.float32)
    A = rng.normal(size=(K, K)).astype(np.float32)
    q, r = np.linalg.qr(A)
    T = (q * np.sign(np.diag(r))[None, :]).astype(np.float32)
    W = rng.uniform(-0.25, 0.25, size=(D, K)).astype(np.float32)
    b = rng.uniform(-0.25, 0.25, size=(D,)).astype(np.float32)
    ref = _host_fallback(p, T, W, b, N)
    act = kernel(pos_initial=p, pos_transition=T, W=W, b=b, sentence_len=N)
    err = np.abs(act - ref).max() / np.abs(ref).max()
    print("max rel err vs host closed form:", err)
